# revision 1
# baseline (speedup 1.0000x reference)
"""GCN regressor on 8 trn2 NeuronCores (raw Bass/bacc kernel).

Sharding: destination-node tiles (128 nodes) are balanced across 8 cores;
edges are partitioned by destination tile so the segment-sum stays local.
Source features are exchanged by AllGather of per-core shards, then fetched
per edge with bulk dma_gather (bf16 rows padded to 256B).

Engine roles per core, per layer:
  gpsimd: bulk dma_gather of source rows (per window x source-bucket piece)
  DVE   : one-hot chunks  oh[e, d] = norm_e * (dst_e == d)
  PE    : segT[f, d] += msg_chunk[e, f]^T @ oh[e, d]  (one PSUM bank group
          per dest tile, tile-major), then a1T @ W2 / a2T @ Wl
  ACT   : psum -> sbuf epilogues (relu, copies)

The gather stream is bucket-major within each window of tiles (so gather
calls stay few and single-bucket); the matmul stream is tile-major (so each
tile's psum accumulation group is contiguous). A static map connects them.
"""
import numpy as np
import ml_dtypes
from contextlib import ExitStack

N, E, IND, HID = 100000, 1000000, 128, 64
M = 8
P = 128
NT = 98                  # dest tiles per core
NW = 14                  # windows
WSZ = NT // NW           # 7 tiles per window
NS = NT * P              # 12544
V = M * NS               # 100352
NBUCK = 4
BSZ = V // NBUCK         # 25088 (int16-safe)
NTILE = (N + P - 1) // P # 782
FE = 128                 # padded row: 256B in bf16
RSEG = 6                 # psum bank ring for tile groups
RO = 32                  # one-hot ring (chunks)
ATR = 16                 # a1T/a2T ring (tiles)

_cache = {}


def _host_prep(x, edge_index, edge_weight):
    ei = np.asarray(edge_index).astype(np.int64)
    ew_in = np.asarray(edge_weight, dtype=np.float64)
    x = np.asarray(x, dtype=np.float32)

    # degrees / norms include the self loops (weight 1)
    src = ei[0]
    dst = ei[1]
    deg = np.bincount(dst, weights=ew_in, minlength=N) + 1.0
    dinv = deg ** -0.5
    norm = (dinv[src] * ew_in * dinv[dst]).astype(np.float32)
    norm_self = (dinv * dinv).astype(np.float32)      # [N]

    tid = dst >> 7
    cnt = np.bincount(tid, minlength=NTILE)
    order = np.argsort(-cnt, kind="stable")

    core_of_t = np.empty(NTILE, np.int64)
    rank_of_t = np.empty(NTILE, np.int64)
    for pi, t in enumerate(order):
        rnd, r = pi // M, pi % M
        core_of_t[t] = r if rnd % 2 == 0 else M - 1 - r
        rank_of_t[t] = rnd
    assign = -np.ones((M, NT), np.int64)
    assign[core_of_t, rank_of_t] = np.arange(NTILE)

    node = np.arange(N, dtype=np.int64)
    tn = node >> 7
    pos_node = core_of_t[tn] * NS + rank_of_t[tn] * P + (node & 127)

    psrc = pos_node[src]
    bucket = psrc // BSZ
    loc = psrc - bucket * BSZ

    ecore = core_of_t[tid]
    erank = rank_of_t[tid]
    ew_w = erank % NW          # window
    ew_s = erank // NW         # slot in window

    # group = (window, slot, bucket) over REAL edges only
    NG = NW * WSZ * NBUCK
    gid = ((ecore * NW + ew_w) * WSZ + ew_s) * NBUCK + bucket
    gcnt = np.bincount(gid, minlength=M * NG).reshape(M, NW, WSZ, NBUCK)
    profile = (-(-gcnt // P)).max(axis=0)          # [NW, WSZ, NBUCK] chunks

    # matmul order: per (w, s): [self chunk, b0.., b1.., b2.., b3..]
    per_tile = profile.sum(axis=2) + 1             # [NW, WSZ] chunks
    CT = int(per_tile.sum())
    # mm-order start of (w, s, b) real group
    mm_start = np.zeros((NW, WSZ, NBUCK), np.int64)
    tile_start = np.zeros((NW, WSZ), np.int64)
    o = 0
    for w in range(NW):
        for s2 in range(WSZ):
            tile_start[w, s2] = o
            o += 1                                  # self chunk
            for b in range(NBUCK):
                mm_start[w, s2, b] = o
                o += int(profile[w, s2, b])
    assert o == CT
    # gather order (real only): (w, b, s)
    g_start = np.zeros((NW, WSZ, NBUCK), np.int64)
    o = 0
    for w in range(NW):
        for b in range(NBUCK):
            for s2 in range(WSZ):
                g_start[w, s2, b] = o
                o += int(profile[w, s2, b])
    CTR = o                                        # real chunks

    sort_idx = np.argsort(gid, kind="stable")
    gs = gid[sort_idx]
    counts = np.bincount(gid, minlength=M * NG)
    starts_sorted = np.zeros(M * NG + 1, np.int64)
    np.cumsum(counts, out=starts_sorted[1:])
    within = np.arange(len(gs), dtype=np.int64) - np.repeat(
        starts_sorted[:-1], counts)
    core_s = gs // NG
    lg = gs - core_s * NG
    pos_mm = mm_start.reshape(-1)[lg] * P + within
    pos_g = g_start.reshape(NW, WSZ, NBUCK).reshape(-1)[lg] * P + within

    L = CT * P
    LR = CTR * P
    idxs = np.zeros((M, LR), np.int16)
    dstl = np.zeros((M, L), np.float32)
    nrm = np.zeros((M, L), np.float32)
    idxs[core_s, pos_g] = loc[sort_idx].astype(np.int16)
    dstl[core_s, pos_mm] = (dst[sort_idx] & 127).astype(np.float32)
    nrm[core_s, pos_mm] = norm[sort_idx]

    # self-chunk columns: dst_local = partition index; nrm = norm_self
    nself = np.zeros((M, NT, P), np.float32)
    for c in range(M):
        for i in range(NT):
            t = assign[c, i]
            if t >= 0:
                lo2 = t * P
                hi2 = min(lo2 + P, N)
                nself[c, i, 0:hi2 - lo2] = norm_self[lo2:hi2]
    pcol = np.arange(P, dtype=np.float32)
    dst3 = dstl.reshape(M, CT, P)
    nrm3 = nrm.reshape(M, CT, P)
    for w in range(NW):
        for s2 in range(WSZ):
            ccs = int(tile_start[w, s2])
            i = w + NW * s2
            dst3[:, ccs, :] = pcol[None, :]
            nrm3[:, ccs, :] = nself[:, i, :]

    idx_w = np.ascontiguousarray(
        np.tile(idxs.reshape(M, LR // 16, 16).transpose(0, 2, 1), (1, 8, 1)))
    dst2 = dst3.transpose(0, 2, 1)
    nrm2 = nrm3.transpose(0, 2, 1)
    cst = np.ascontiguousarray(np.concatenate(
        [dst2, nrm2], axis=2)).astype(np.float32)

    xpad = np.vstack([x, np.zeros((NTILE * P - N + P, IND), np.float32)])
    ntab = np.where(assign >= 0, assign, NTILE)[:, :, None] * P + np.arange(P)
    xt = xpad[ntab.reshape(M, NS)]
    xt = np.ascontiguousarray(xt.transpose(0, 2, 1)).astype(ml_dtypes.bfloat16)

    return dict(idx_w=idx_w, cst=cst, xt=xt, assign=assign,
                profile=profile, CT=CT)


def _build_nc(profile, CT, reps=1):
    import concourse.bacc as bacc
    import concourse.mybir as mybir
    from concourse.library_config import mlp

    # gather pieces: (w, b) -> chunks, and gather-order window starts
    piece_wb = profile.sum(axis=1)                     # [NW, NBUCK]
    win_chunks = piece_wb.sum(axis=1)                  # [NW]
    win_start = np.zeros(NW + 1, np.int64)
    np.cumsum(win_chunks, out=win_start[1:])
    MAXW = int(win_chunks.max())
    # gather-order offset of (w, b) piece within its window
    pboff = np.zeros((NW, NBUCK), np.int64)
    for w in range(NW):
        o = 0
        for b in range(NBUCK):
            pboff[w, b] = o
            o += int(piece_wb[w, b])
    # gather-order offset of (w, s, b) group within window
    gwoff = np.zeros((NW, WSZ, NBUCK), np.int64)
    for w in range(NW):
        for b in range(NBUCK):
            o = pboff[w, b]
            for s in range(WSZ):
                gwoff[w, s, b] = o
                o += int(profile[w, s, b])

    # matmul-order schedule: chunk -> (w, s, b, c); b == -1 is the
    # self-loop chunk (lhsT comes from the local gbuf shard, no gather)
    sched = []
    for w in range(NW):
        for s in range(WSZ):
            sched.append((w, s, -1, 0))
            for b in range(NBUCK):
                for c in range(int(profile[w, s, b])):
                    sched.append((w, s, b, c))
    NCH = len(sched)
    assert NCH == CT
    CTR = CT - NW * WSZ
    first_chunk, last_chunk = {}, {}
    for cc, (w, s, b, c) in enumerate(sched):
        first_chunk.setdefault((w, s), cc)
        last_chunk[(w, s)] = cc
    for w in range(NW):
        for s in range(WSZ):
            assert (w, s) in first_chunk, "empty tile"
    # matmul-order chunk count through window w (for gather WAR)
    mm_win_end = np.zeros(NW + 1, np.int64)
    for cc, (w, s, b, c) in enumerate(sched):
        mm_win_end[w + 1] = cc + 1

    nc = bacc.Bacc("TRN2")
    xtd = nc.dram_tensor("xt", [P, NS], mybir.dt.bfloat16, kind="ExternalInput")
    idxd = nc.dram_tensor("idx_w", [P, CTR * 8], mybir.dt.int16,
                          kind="ExternalInput")
    cstd = nc.dram_tensor("cst", [P, 2 * CT], mybir.dt.float32,
                          kind="ExternalInput")
    iotad = nc.dram_tensor("iotab", [P, P], mybir.dt.bfloat16,
                           kind="ExternalInput")
    w1d = nc.dram_tensor("w1", [IND, HID], mybir.dt.bfloat16,
                         kind="ExternalInput")
    w2d = nc.dram_tensor("w2", [HID, HID], mybir.dt.bfloat16,
                         kind="ExternalInput")
    wld = nc.dram_tensor("wl", [HID, 1], mybir.dt.bfloat16,
                         kind="ExternalInput")
    outd = nc.dram_tensor("outd", [P, NT], mybir.dt.float32,
                          kind="ExternalOutput")

    g1_shard = nc.dram_tensor("g1_shard", [NS, FE], mybir.dt.bfloat16)
    g2_shard = nc.dram_tensor("g2_shard", [NS, FE], mybir.dt.bfloat16)
    g1_full = nc.dram_tensor("g1_full", [V, FE], mybir.dt.bfloat16,
                             addr_space="Shared")
    g2_full = nc.dram_tensor("g2_full", [V, FE], mybir.dt.bfloat16,
                             addr_space="Shared")
    tables = (g1_full, g2_full)

    with ExitStack() as ctx:
        sb = lambda nm, sh, dt: ctx.enter_context(nc.sbuf_tensor(nm, sh, dt))
        sem = lambda nm: ctx.enter_context(nc.semaphore(nm))

        xt_t = sb("xt_t", [P, NS], mybir.dt.bfloat16)
        idx_t = sb("idx_t", [P, CTR * 8], mybir.dt.int16)
        cst = sb("cst_t", [P, 2 * CT], mybir.dt.float32)
        iota_t = sb("iota_t", [P, P], mybir.dt.bfloat16)
        w1_t = sb("w1_t", [IND, HID], mybir.dt.bfloat16)
        w2_t = sb("w2_t", [HID, HID], mybir.dt.bfloat16)
        wl_t = sb("wl_t", [HID, 1], mybir.dt.bfloat16)
        msg_ring = sb("msg_ring", [P, 2, MAXW, FE], mybir.dt.bfloat16)
        oh_ring = sb("oh_ring", [P, RO, P], mybir.dt.bfloat16)
        # g2_buf aliases xt (xt is dead once phase A matmuls finish; the
        # first g2 write is sem-ordered after them via mm2 -> relu -> seg)
        gbuf = [sb("g1_buf", [P, NT, FE], mybir.dt.bfloat16),
                xt_t[:].rearrange("p (i f) -> p i f", f=FE)]
        at_ring = sb("at_ring", [HID, ATR, P], mybir.dt.bfloat16)
        out_buf = sb("out_buf", [P, NT], mybir.dt.float32)

        # one full 2KB bank per concurrent accumulation slot: matmul start
        # zeroes the whole bank-granular zero region on HW
        seg_ps = ctx.enter_context(
            nc.psum_tensor("seg_ps", [HID, RSEG, 512], mybir.dt.float32))
        misc_ps = ctx.enter_context(
            nc.psum_tensor("misc_ps", [P, 2, 512], mybir.dt.float32))
        ga_ps = [misc_ps[:, 0, 0:HID], misc_ps[:, 1, 0:HID]]

        s_in = sem("s_in")
        s_mma = sem("s_mma")
        s_g1c = sem("s_g1c")
        s_shard = [sem("s_shard0"), sem("s_shard1")]
        s_cc = sem("s_cc")
        # per (slot parity, bucket) gather sems, per layer
        s_msgs = [[sem(f"s_msg{l}_{i}") for i in range(2 * NBUCK)]
                  for l in (0, 1)]
        s_oh = [sem("s_oh0"), sem("s_oh1")]
        s_mm = [sem("s_mm0"), sem("s_mm1")]
        s_ep = [sem("s_ep0"), sem("s_ep1")]
        s_mm2 = [sem("s_mm20"), sem("s_mm21")]
        s_epc = [sem("s_epc0"), sem("s_epc1")]
        s_fin = sem("s_fin")
        s_z = sem("s_z")

        with nc.Block() as block:

            @block.sync
            def _(sync):
                sync.dma_start(idx_t[:], idxd[:]).then_inc(s_in, 16)
                sync.dma_start(cst[:], cstd[:]).then_inc(s_in, 16)
                sync.dma_start(iota_t[:], iotad[:]).then_inc(s_in, 16)
                sync.dma_start(xt_t[:], xtd[:]).then_inc(s_in, 16)
                sync.dma_start(w1_t[:], w1d[:]).then_inc(s_in, 16)
                sync.dma_start(w2_t[:], w2d[:]).then_inc(s_in, 16)
                sync.dma_start(wl_t[:], wld[:]).then_inc(s_in, 16)
                sync.wait_ge(s_z, 1)
                for rep in range(reps):
                    sync.wait_ge(s_g1c, (rep + 1) * NT)
                    sync.dma_start(
                        g1_shard[:].rearrange("(i p) f -> p i f", p=P),
                        gbuf[0][:]).then_inc(s_shard[0], 16)
                    if rep == 0:
                        sync.wait_ge(s_z, 2)
                    sync.wait_ge(s_epc[0], (rep + 1) * NT)
                    sync.dma_start(
                        g2_shard[:].rearrange("(i p) f -> p i f", p=P),
                        gbuf[1][:]).then_inc(s_shard[1], 16)
                    sync.wait_ge(s_epc[1], (rep + 1) * NT)
                    sync.dma_start(outd[:], out_buf[:]).then_inc(s_fin, 16)
                sync.wait_ge(s_fin, 16 * reps)

            @block.gpsimd
            def _(gpsimd):
                gpsimd.load_library(mlp)
                gpsimd.wait_ge(s_in, 112)
                GMAX = 8   # max chunks per dma_gather call (1024 idxs: HW limit)
                for rep in range(reps):
                  gpsimd.wait_ge(s_shard[0], 16 * (rep + 1))
                  gpsimd.collective_compute(
                      "AllGather", mybir.AluOpType.bypass,
                      ins=[g1_shard[:]], outs=[g1_full[:]],
                      replica_groups=[list(range(M))],
                  ).then_inc(s_cc, 1)
                  for l in (0, 1):
                    bmm = [rep * NCH, rep * NCH]
                    gpsimd.wait_ge(s_cc, 2 * rep + l + 1)
                    if l == 0 and rep > 0:
                        # msg ring handoff from previous rep's layer 1
                        gpsimd.wait_ge(s_mm[1], rep * NCH)
                    for w in range(NW):
                        if w >= 2:
                            gpsimd.wait_ge(
                                s_mm[l], rep * NCH + int(mm_win_end[w - 1]))
                        for b in range(NBUCK):
                            pc = int(piece_wb[w, b])
                            if pc == 0:
                                continue
                            for q0 in range(0, pc, GMAX):
                                qc = min(GMAX, pc - q0)
                                lo = int(win_start[w] + pboff[w, b]) + q0
                                n = qc * P
                                gpsimd.dma_gather(
                                    msg_ring[:, w % 2,
                                             int(pboff[w, b]) + q0:
                                             int(pboff[w, b]) + q0 + qc, :],
                                    tables[l][b * BSZ:(b + 1) * BSZ, :],
                                    idx_t[:, lo * 8:lo * 8 + n // 16],
                                    n, n, FE,
                                ).then_inc(s_msgs[l][(w % 2) * NBUCK + b], 16)
                    if l == 0:
                        gpsimd.wait_ge(s_shard[1], 16 * (rep + 1))
                        gpsimd.collective_compute(
                            "AllGather", mybir.AluOpType.bypass,
                            ins=[g2_shard[:]], outs=[g2_full[:]],
                            replica_groups=[list(range(M))],
                        ).then_inc(s_cc, 1)

            @block.vector
            def _(vector):
                vector.memset(gbuf[0][:, :, HID:FE], 0).then_inc(s_z, 1)
                vector.wait_ge(s_in, 112)
                # g2_buf aliases xt: zero its pad columns only after phase A
                vector.wait_ge(s_mma, NT)
                vector.memset(gbuf[1][:, :, HID:FE], 0).then_inc(s_z, 1)
                for rep in range(reps):
                  for l in (0, 1):
                    if l == 1:
                        # oh ring handoff between layers
                        vector.wait_ge(s_mm[0], (rep + 1) * NCH)
                    elif rep > 0:
                        vector.wait_ge(s_mm[1], rep * NCH)
                    ndone = 0
                    for cc in range(NCH):
                        if cc % 16 == 0 and cc + 16 > RO:
                            vector.wait_ge(
                                s_mm[l], rep * NCH + min(cc + 16, NCH) - RO)
                        ts = vector.tensor_scalar(
                            out=oh_ring[:, cc % RO, :], in0=iota_t[:],
                            scalar1=cst[:, cc:cc + 1],
                            scalar2=cst[:, CT + cc:CT + cc + 1],
                            op0=mybir.AluOpType.is_equal,
                            op1=mybir.AluOpType.mult,
                        )
                        if cc % 8 == 7 or cc == NCH - 1:
                            ts.then_inc(s_oh[l], cc + 1 - ndone)
                            ndone = cc + 1

            @block.scalar
            def _(scalar):
                Relu = mybir.ActivationFunctionType.Relu
                Copy = mybir.ActivationFunctionType.Copy
                scalar.wait_ge(s_in, 112)
                for rep in range(reps):

                  for i in range(NT):
                    scalar.wait_ge(s_mma, rep * NT + i + 1)
                    scalar.activation(
                        out=gbuf[0][:, i, 0:HID], in_=ga_ps[i % 2], func=Copy,
                    ).then_inc(s_g1c, 1)

                  def relus(l, w, rep=rep):
                    for s in range(WSZ):
                        g = w * WSZ + s
                        scalar.wait_ge(
                            s_mm[l], rep * NCH + last_chunk[(w, s)] + 1)
                        if g >= ATR:
                            scalar.wait_ge(s_mm2[l], rep * NT + g - ATR + 1)
                        scalar.activation(
                            out=at_ring[:, g % ATR, :],
                            in_=seg_ps[:, g % RSEG, 0:P],
                            func=Relu,
                        ).then_inc(s_ep[l], 1)

                  def copies(l, w, rep=rep):
                    for s in range(WSZ):
                        g = w * WSZ + s
                        i = w + NW * s
                        scalar.wait_ge(s_mm2[l], rep * NT + g + 1)
                        if l == 0:
                            scalar.activation(
                                out=gbuf[1][:, i, 0:HID], in_=ga_ps[g % 2],
                                func=Copy,
                            ).then_inc(s_epc[0], 1)
                        else:
                            scalar.activation(
                                out=out_buf[:, i:i + 1],
                                in_=ga_ps[g % 2][:, 0:1], func=Copy,
                            ).then_inc(s_epc[1], 1)

                  for l in (0, 1):
                    if l == 0 and rep > 0:
                        scalar.wait_ge(s_mm2[1], rep * NT)
                    for w in range(NW):
                        relus(l, w)
                        if w >= 1:
                            copies(l, w - 1)
                    copies(l, NW - 1)

            @block.tensor
            def _(tensor):
                tensor.wait_ge(s_in, 112)
                GMAX = 8
                # per-layer total gather sub-calls per (parity, bucket) sem
                NCALLS = [0] * (2 * NBUCK)
                for w in range(NW):
                    for b in range(NBUCK):
                        pc = int(piece_wb[w, b])
                        NCALLS[(w % 2) * NBUCK + b] += (pc + GMAX - 1) // GMAX
                for rep in range(reps):
                  if rep > 0:
                    tensor.wait_ge(s_epc[1], rep * NT)
                    tensor.wait_ge(s_ep[1], rep * NT)
                  for i in range(NT):
                    gi = rep * NT + i
                    if gi >= 2:
                        tensor.wait_ge(s_g1c, gi - 1)
                    tensor.matmul(
                        out=ga_ps[i % 2],
                        lhsT=xt_t[:, i * P:(i + 1) * P],
                        rhs=w1_t[:, :],
                        start=True, stop=True,
                    ).then_inc(s_mma, 1)

                  def mm2s(l, w, rep=rep):
                    for s in range(WSZ):
                        g = w * WSZ + s
                        tensor.wait_ge(s_ep[l], rep * NT + g + 1)
                        if g >= 2:
                            tensor.wait_ge(s_epc[l], rep * NT + g - 1)
                        tensor.matmul(
                            out=ga_ps[g % 2] if l == 0 else ga_ps[g % 2][:, 0:1],
                            lhsT=at_ring[:, g % ATR, :],
                            rhs=w2_t[:, :] if l == 0 else wl_t[:, :],
                            start=True, stop=True,
                        ).then_inc(s_mm2[l], 1)

                  for l in (0, 1):
                    if l == 0:
                        tensor.wait_ge(s_g1c, (rep + 1) * NT)
                    else:
                        tensor.wait_ge(s_epc[0], (rep + 1) * NT)
                    ncall = [0] * (2 * NBUCK)
                    waited = set()
                    for cc, (w, s, b, c) in enumerate(sched):
                        g = w * WSZ + s
                        if b >= 0 and (w, b) not in waited:
                            waited.add((w, b))
                            pc = int(piece_wb[w, b])
                            if pc > 0:
                                sl = (w % 2) * NBUCK + b
                                ncall[sl] += (pc + GMAX - 1) // GMAX
                                tensor.wait_ge(
                                    s_msgs[l][sl],
                                    (rep * NCALLS[sl] + ncall[sl]) * 16)
                        if cc % 8 == 0:
                            tensor.wait_ge(
                                s_oh[l], rep * NCH + min(cc + 8, NCH))
                        st = first_chunk[(w, s)] == cc
                        if st and g >= RSEG:
                            tensor.wait_ge(s_ep[l], rep * NT + g - RSEG + 1)
                        if b < 0:
                            lhs = gbuf[l][:, w + NW * s, 0:HID]
                        else:
                            moff = int(gwoff[w, s, b]) + c
                            lhs = msg_ring[:, w % 2, moff, 0:HID]
                        tensor.matmul(
                            out=seg_ps[:, g % RSEG, 0:P],
                            lhsT=lhs,
                            rhs=oh_ring[:, cc % RO, :],
                            start=st, stop=last_chunk[(w, s)] == cc,
                        ).then_inc(s_mm[l], 1)
                        if cc == int(mm_win_end[w + 1]) - 1 and w >= 1:
                            mm2s(l, w - 1)
                    mm2s(l, NW - 1)

    nc.compile()
    return nc


def _run(nc, in_maps):
    from concourse.bass_utils import run_bass_kernel_spmd
    return run_bass_kernel_spmd(nc, in_maps, core_ids=list(range(M)))


class _Runner:
    """Persistent compiled executable + device-resident inputs.

    Mimics bass2jax.run_bass_via_pjrt but keeps the jitted shard_map and the
    device arrays alive so repeat executions only pay dispatch + exec.
    """

    def __init__(self, nc, in_maps):
        import jax
        import numpy as _np
        import concourse.mybir as mybir
        from jax.sharding import Mesh, PartitionSpec
        from jax.experimental.shard_map import shard_map
        from concourse import bass2jax

        bass2jax.install_neuronx_cc_hook()
        self.jax = jax
        pname = nc.partition_id_tensor.name if nc.partition_id_tensor else None
        in_names, out_names, out_avals, zero_outs = [], [], [], []
        for alloc in nc.m.functions[0].allocations:
            if not isinstance(alloc, mybir.MemoryLocationSet):
                continue
            name = alloc.memorylocations[0].name
            if alloc.kind == "ExternalInput":
                if name != pname:
                    in_names.append(name)
            elif alloc.kind == "ExternalOutput":
                shape = tuple(alloc.tensor_shape)
                dtype = mybir.dt.np(alloc.dtype)
                out_names.append(name)
                out_avals.append(jax.core.ShapedArray(shape, dtype))
                zero_outs.append(_np.zeros(shape, dtype))
        n_params = len(in_names)
        all_in = in_names + out_names
        if pname is not None:
            all_in.append(pname)
        self.out_names = out_names
        self.out_avals = out_avals

        def _body(*args):
            operands = list(args)
            if pname is not None:
                operands.append(bass2jax.partition_id_tensor())
            return tuple(bass2jax._bass_exec_p.bind(
                *operands, out_avals=tuple(out_avals), in_names=tuple(all_in),
                out_names=tuple(out_names), lowering_input_output_aliases=(),
                sim_require_finite=True, sim_require_nnan=True, nc=nc))

        devices = jax.devices()[:M]
        mesh = Mesh(_np.asarray(devices), ("core",))
        nio = n_params + len(out_names)
        self.fn = jax.jit(shard_map(
            _body, mesh=mesh, in_specs=(PartitionSpec("core"),) * nio,
            out_specs=(PartitionSpec("core"),) * len(out_names),
            check_rep=False),
            donate_argnums=tuple(range(n_params, nio)), keep_unused=True)
        self.sharding = jax.sharding.NamedSharding(mesh, PartitionSpec("core"))
        concat_in = [
            _np.concatenate([_np.asarray(in_maps[c][nm]) for c in range(M)],
                            axis=0)
            for nm in in_names]
        self.dev_in = [jax.device_put(a, self.sharding) for a in concat_in]
        self.zero_outs = zero_outs

    def __call__(self):
        zs = [self.jax.device_put(
            self.jax.numpy.zeros((M * z.shape[0], *z.shape[1:]), z.dtype),
            self.sharding) for z in self.zero_outs]
        outs = self.fn(*self.dev_in, *zs)
        self.jax.block_until_ready(outs)
        return outs

    def results(self):
        import numpy as _np
        outs = self()
        return [
            {nm: _np.asarray(outs[i]).reshape(
                M, *self.out_avals[i].shape)[c]
             for i, nm in enumerate(self.out_names)}
            for c in range(M)]


def kernel(x, edge_index, edge_weight, W1, b1, W2, b2, Wl, bl):
    W1 = np.asarray(W1, np.float32)
    b1 = np.asarray(b1, np.float32)
    W2 = np.asarray(W2, np.float32)
    b2 = np.asarray(b2, np.float32)
    Wl = np.asarray(Wl, np.float32)
    bl = np.asarray(bl, np.float32)
    if np.any(b1 != 0) or np.any(b2 != 0):
        return _kernel_numpy(x, edge_index, edge_weight, W1, b1, W2, b2, Wl, bl)

    prep = _host_prep(x, edge_index, edge_weight)
    in_maps = _in_maps(prep, W1, W2, Wl)
    nc = _get_nc(prep)
    res = _run(nc, in_maps)
    return _assemble(res.results, prep, bl)


def _get_nc(prep, reps=1):
    key = (prep["CT"], reps, prep["profile"].tobytes())
    if key not in _cache:
        _cache[key] = _build_nc(prep["profile"], prep["CT"], reps=reps)
    return _cache[key]


def _in_maps(prep, W1, W2, Wl):
    w1b = np.ascontiguousarray(W1).astype(ml_dtypes.bfloat16)
    w2b = np.ascontiguousarray(W2).astype(ml_dtypes.bfloat16)
    wlb = np.ascontiguousarray(Wl).astype(ml_dtypes.bfloat16)
    iota_b = np.tile(np.arange(P, dtype=np.float32), (P, 1)).astype(
        ml_dtypes.bfloat16)
    return [{
        "xt": prep["xt"][c], "idx_w": prep["idx_w"][c], "cst": prep["cst"][c],
        "iotab": iota_b, "w1": w1b, "w2": w2b, "wl": wlb,
    } for c in range(M)]


def _assemble(results, prep, bl):
    out_full = np.zeros(NTILE * P + P, np.float32)
    assign = prep["assign"]
    for c in range(M):
        o = np.asarray(results[c]["outd"])
        for i in range(NT):
            t = assign[c, i]
            if t >= 0:
                out_full[t * P:(t + 1) * P] = o[:, i]
    return (out_full[:N] + np.float32(np.asarray(bl).reshape(-1)[0])).astype(
        np.float32)


def _kernel_numpy(x, edge_index, edge_weight, W1, b1, W2, b2, Wl, bl):
    x = np.asarray(x, dtype=np.float32)
    ei = np.asarray(edge_index).astype(np.int64)
    ew_in = np.asarray(edge_weight, dtype=np.float32)
    loop = np.arange(N, dtype=np.int64)
    src = np.concatenate([ei[0], loop])
    dst = np.concatenate([ei[1], loop])
    ew = np.concatenate([ew_in, np.ones(N, dtype=np.float32)])
    deg = np.bincount(dst, weights=ew, minlength=N)
    dinv = np.where(deg > 0, 1.0 / np.sqrt(deg), 0.0).astype(np.float32)
    norm = (dinv[src] * ew * dinv[dst]).astype(np.float32)

    def prop(h):
        msg = h[src] * norm[:, None]
        out = np.zeros((N, h.shape[1]), np.float32)
        np.add.at(out, dst, msg)
        return out

    h = np.maximum(prop(x @ W1) + b1, 0.0)
    h = np.maximum(prop(h @ W2) + b2, 0.0)
    return (h @ Wl + bl).squeeze(-1).astype(np.float32)



# revision 5
# speedup vs baseline: 5.1771x; 5.1771x over previous
"""GCN regressor on 8 trn2 NeuronCores (raw Bass/bacc kernel).

Sharding: destination-node tiles (128 nodes) are balanced across 8 cores;
edges are partitioned by destination tile so the segment-sum stays local.
Source features are exchanged by AllGather of per-core shards, then fetched
per edge with bulk dma_gather (bf16 rows padded to 256B).

Engine roles per core, per layer:
  gpsimd: bulk dma_gather of source rows (per window x source-bucket piece)
  DVE   : one-hot chunks  oh[e, d] = norm_e * (dst_e == d)
  PE    : segT[f, d] += msg_chunk[e, f]^T @ oh[e, d]  (one PSUM bank group
          per dest tile, tile-major), then a1T @ W2 / a2T @ Wl
  ACT   : psum -> sbuf epilogues (relu, copies)

The gather stream is bucket-major within each window of tiles (so gather
calls stay few and single-bucket); the matmul stream is tile-major (so each
tile's psum accumulation group is contiguous). A static map connects them.
"""
import numpy as np
import ml_dtypes
from contextlib import ExitStack

N, E, IND, HID = 100000, 1000000, 128, 64
M = 8
P = 128
NT = 98                  # dest tiles per core
NW = 14                  # windows
WSZ = NT // NW           # 7 tiles per window
NS = NT * P              # 12544
V = M * NS               # 100352
NBUCK = 4
BSZ = V // NBUCK         # 25088 (int16-safe)
NTILE = (N + P - 1) // P # 782
FE = 128                 # padded row: 256B in bf16
RSEG = 6                 # psum bank ring for tile groups
RO = 32                  # one-hot ring (chunks)
ATR = 16                 # a1T/a2T ring (tiles)

_cache = {}


def _host_prep(x, edge_index, edge_weight):
    ei = np.asarray(edge_index).astype(np.int64)
    ew_in = np.asarray(edge_weight, dtype=np.float64)
    x = np.asarray(x, dtype=np.float32)

    # degrees / norms include the self loops (weight 1)
    src = ei[0]
    dst = ei[1]
    deg = np.bincount(dst, weights=ew_in, minlength=N) + 1.0
    dinv = deg ** -0.5
    norm = (dinv[src] * ew_in * dinv[dst]).astype(np.float32)
    norm_self = (dinv * dinv).astype(np.float32)      # [N]

    tid = dst >> 7
    cnt = np.bincount(tid, minlength=NTILE)
    order = np.argsort(-cnt, kind="stable")

    core_of_t = np.empty(NTILE, np.int64)
    rank_of_t = np.empty(NTILE, np.int64)
    for pi, t in enumerate(order):
        rnd, r = pi // M, pi % M
        core_of_t[t] = r if rnd % 2 == 0 else M - 1 - r
        rank_of_t[t] = rnd
    assign = -np.ones((M, NT), np.int64)
    assign[core_of_t, rank_of_t] = np.arange(NTILE)

    node = np.arange(N, dtype=np.int64)
    tn = node >> 7
    pos_node = core_of_t[tn] * NS + rank_of_t[tn] * P + (node & 127)

    psrc = pos_node[src]
    bucket = psrc // BSZ
    loc = psrc - bucket * BSZ

    ecore = core_of_t[tid]
    erank = rank_of_t[tid]
    ew_w = erank % NW          # window
    ew_s = erank // NW         # slot in window

    # group = (window, slot, bucket) over REAL edges only
    NG = NW * WSZ * NBUCK
    gid = ((ecore * NW + ew_w) * WSZ + ew_s) * NBUCK + bucket
    gcnt = np.bincount(gid, minlength=M * NG).reshape(M, NW, WSZ, NBUCK)
    profile = (-(-gcnt // P)).max(axis=0)          # [NW, WSZ, NBUCK] chunks

    # matmul order: per (w, s): [self chunk, b0.., b1.., b2.., b3..]
    per_tile = profile.sum(axis=2) + 1             # [NW, WSZ] chunks
    CT = int(per_tile.sum())
    # mm-order start of (w, s, b) real group
    mm_start = np.zeros((NW, WSZ, NBUCK), np.int64)
    tile_start = np.zeros((NW, WSZ), np.int64)
    o = 0
    for w in range(NW):
        for s2 in range(WSZ):
            tile_start[w, s2] = o
            o += 1                                  # self chunk
            for b in range(NBUCK):
                mm_start[w, s2, b] = o
                o += int(profile[w, s2, b])
    assert o == CT
    # gather order (real only): (w, b, s)
    g_start = np.zeros((NW, WSZ, NBUCK), np.int64)
    o = 0
    for w in range(NW):
        for b in range(NBUCK):
            for s2 in range(WSZ):
                g_start[w, s2, b] = o
                o += int(profile[w, s2, b])
    CTR = o                                        # real chunks

    sort_idx = np.argsort(gid, kind="stable")
    gs = gid[sort_idx]
    counts = np.bincount(gid, minlength=M * NG)
    starts_sorted = np.zeros(M * NG + 1, np.int64)
    np.cumsum(counts, out=starts_sorted[1:])
    within = np.arange(len(gs), dtype=np.int64) - np.repeat(
        starts_sorted[:-1], counts)
    core_s = gs // NG
    lg = gs - core_s * NG
    pos_mm = mm_start.reshape(-1)[lg] * P + within
    pos_g = g_start.reshape(NW, WSZ, NBUCK).reshape(-1)[lg] * P + within

    L = CT * P
    LR = CTR * P
    idxs = np.zeros((M, LR), np.int16)
    dstl = np.zeros((M, L), np.float32)
    nrm = np.zeros((M, L), np.float32)
    idxs[core_s, pos_g] = loc[sort_idx].astype(np.int16)
    dstl[core_s, pos_mm] = (dst[sort_idx] & 127).astype(np.float32)
    nrm[core_s, pos_mm] = norm[sort_idx]

    # self-chunk columns: dst_local = partition index; nrm = norm_self
    nself = np.zeros((M, NT, P), np.float32)
    for c in range(M):
        for i in range(NT):
            t = assign[c, i]
            if t >= 0:
                lo2 = t * P
                hi2 = min(lo2 + P, N)
                nself[c, i, 0:hi2 - lo2] = norm_self[lo2:hi2]
    pcol = np.arange(P, dtype=np.float32)
    dst3 = dstl.reshape(M, CT, P)
    nrm3 = nrm.reshape(M, CT, P)
    for w in range(NW):
        for s2 in range(WSZ):
            ccs = int(tile_start[w, s2])
            i = w + NW * s2
            dst3[:, ccs, :] = pcol[None, :]
            nrm3[:, ccs, :] = nself[:, i, :]

    idx_w = np.ascontiguousarray(
        np.tile(idxs.reshape(M, LR // 16, 16).transpose(0, 2, 1), (1, 8, 1)))
    dst2 = dst3.transpose(0, 2, 1)
    nrm2 = nrm3.transpose(0, 2, 1)
    cst = np.ascontiguousarray(np.concatenate(
        [dst2, nrm2], axis=2)).astype(np.float32)

    xpad = np.vstack([x, np.zeros((NTILE * P - N + P, IND), np.float32)])
    ntab = np.where(assign >= 0, assign, NTILE)[:, :, None] * P + np.arange(P)
    xt = xpad[ntab.reshape(M, NS)]
    xt = np.ascontiguousarray(xt.transpose(0, 2, 1)).astype(ml_dtypes.bfloat16)

    return dict(idx_w=idx_w, cst=cst, xt=xt, assign=assign,
                profile=profile, CT=CT)


def _build_nc(profile, CT, reps=1):
    import os
    import concourse.bacc as bacc
    import concourse.mybir as mybir
    from concourse.library_config import mlp
    _abl = os.environ.get("KABL", "")

    # gather pieces: (w, b) -> chunks, and gather-order window starts
    piece_wb = profile.sum(axis=1)                     # [NW, NBUCK]
    win_chunks = piece_wb.sum(axis=1)                  # [NW]
    win_start = np.zeros(NW + 1, np.int64)
    np.cumsum(win_chunks, out=win_start[1:])
    MAXW = int(win_chunks.max())
    # gather-order offset of (w, b) piece within its window
    pboff = np.zeros((NW, NBUCK), np.int64)
    for w in range(NW):
        o = 0
        for b in range(NBUCK):
            pboff[w, b] = o
            o += int(piece_wb[w, b])
    # gather-order offset of (w, s, b) group within window
    gwoff = np.zeros((NW, WSZ, NBUCK), np.int64)
    for w in range(NW):
        for b in range(NBUCK):
            o = pboff[w, b]
            for s in range(WSZ):
                gwoff[w, s, b] = o
                o += int(profile[w, s, b])

    # matmul-order schedule: chunk -> (w, s, b, c); b == -1 is the
    # self-loop chunk (lhsT comes from the local gbuf shard, no gather)
    sched = []
    for w in range(NW):
        for s in range(WSZ):
            sched.append((w, s, -1, 0))
            for b in range(NBUCK):
                for c in range(int(profile[w, s, b])):
                    sched.append((w, s, b, c))
    NCH = len(sched)
    assert NCH == CT
    CTR = CT - NW * WSZ
    first_chunk, last_chunk = {}, {}
    for cc, (w, s, b, c) in enumerate(sched):
        first_chunk.setdefault((w, s), cc)
        last_chunk[(w, s)] = cc
    for w in range(NW):
        for s in range(WSZ):
            assert (w, s) in first_chunk, "empty tile"
    # matmul-order chunk count through window w (for gather WAR)
    mm_win_end = np.zeros(NW + 1, np.int64)
    for cc, (w, s, b, c) in enumerate(sched):
        mm_win_end[w + 1] = cc + 1

    nc = bacc.Bacc("TRN2")
    xtd = nc.dram_tensor("xt", [P, NS], mybir.dt.bfloat16, kind="ExternalInput")
    idxd = nc.dram_tensor("idx_w", [P, CTR * 8], mybir.dt.int16,
                          kind="ExternalInput")
    cstd = nc.dram_tensor("cst", [P, 2 * CT], mybir.dt.float32,
                          kind="ExternalInput")
    iotad = nc.dram_tensor("iotab", [P, P], mybir.dt.bfloat16,
                           kind="ExternalInput")
    w1d = nc.dram_tensor("w1", [IND, HID], mybir.dt.bfloat16,
                         kind="ExternalInput")
    w2d = nc.dram_tensor("w2", [HID, HID], mybir.dt.bfloat16,
                         kind="ExternalInput")
    wld = nc.dram_tensor("wl", [HID, 1], mybir.dt.bfloat16,
                         kind="ExternalInput")
    outd = nc.dram_tensor("outd", [P, NT], mybir.dt.float32,
                          kind="ExternalOutput")

    g1_shard = nc.dram_tensor("g1_shard", [NS, FE], mybir.dt.bfloat16)
    g2_shard = nc.dram_tensor("g2_shard", [NS, FE], mybir.dt.bfloat16)
    g1_full = nc.dram_tensor("g1_full", [V, FE], mybir.dt.bfloat16,
                             addr_space="Shared")
    g2_full = nc.dram_tensor("g2_full", [V, FE], mybir.dt.bfloat16,
                             addr_space="Shared")
    tables = (g1_full, g2_full)

    with ExitStack() as ctx:
        sb = lambda nm, sh, dt: ctx.enter_context(nc.sbuf_tensor(nm, sh, dt))
        sem = lambda nm: ctx.enter_context(nc.semaphore(nm))

        xt_t = sb("xt_t", [P, NS], mybir.dt.bfloat16)
        idx_t = sb("idx_t", [P, CTR * 8], mybir.dt.int16)
        cst = sb("cst_t", [P, 2 * CT], mybir.dt.float32)
        iota_t = sb("iota_t", [P, P], mybir.dt.bfloat16)
        w1_t = sb("w1_t", [IND, HID], mybir.dt.bfloat16)
        w2_t = sb("w2_t", [HID, HID], mybir.dt.bfloat16)
        wl_t = sb("wl_t", [HID, 1], mybir.dt.bfloat16)
        msg_ring = sb("msg_ring", [P, 2, MAXW, FE], mybir.dt.bfloat16)
        oh_ring = sb("oh_ring", [P, RO, P], mybir.dt.bfloat16)
        # g2_buf aliases xt (xt is dead once phase A matmuls finish; the
        # first g2 write is sem-ordered after them via mm2 -> relu -> seg)
        gbuf = [sb("g1_buf", [P, NT, FE], mybir.dt.bfloat16),
                xt_t[:].rearrange("p (i f) -> p i f", f=FE)]
        at_ring = sb("at_ring", [HID, ATR, P], mybir.dt.bfloat16)
        out_buf = sb("out_buf", [P, NT], mybir.dt.float32)

        # one full 2KB bank per concurrent accumulation slot: matmul start
        # zeroes the whole bank-granular zero region on HW
        seg_ps = ctx.enter_context(
            nc.psum_tensor("seg_ps", [HID, RSEG, 512], mybir.dt.float32))
        misc_ps = ctx.enter_context(
            nc.psum_tensor("misc_ps", [P, 2, 512], mybir.dt.float32))
        ga_ps = [misc_ps[:, 0, 0:HID], misc_ps[:, 1, 0:HID]]

        s_in = sem("s_in")
        s_mma = sem("s_mma")
        s_g1c = sem("s_g1c")
        s_shard = [sem("s_shard0"), sem("s_shard1")]
        s_cc = sem("s_cc")
        # per (slot parity, bucket) gather sems, per layer
        s_msgs = [[sem(f"s_msg{l}_{i}") for i in range(2 * NBUCK)]
                  for l in (0, 1)]
        s_oh = [sem("s_oh0"), sem("s_oh1")]
        s_mm = [sem("s_mm0"), sem("s_mm1")]
        s_ep = [sem("s_ep0"), sem("s_ep1")]
        s_mm2 = [sem("s_mm20"), sem("s_mm21")]
        s_epc = [sem("s_epc0"), sem("s_epc1")]
        s_fin = sem("s_fin")
        s_z = sem("s_z")

        with nc.Block() as block:

            @block.sync
            def _(sync):
                sync.dma_start(idx_t[:], idxd[:]).then_inc(s_in, 16)
                sync.dma_start(cst[:], cstd[:]).then_inc(s_in, 16)
                sync.dma_start(iota_t[:], iotad[:]).then_inc(s_in, 16)
                sync.dma_start(xt_t[:], xtd[:]).then_inc(s_in, 16)
                sync.dma_start(w1_t[:], w1d[:]).then_inc(s_in, 16)
                sync.dma_start(w2_t[:], w2d[:]).then_inc(s_in, 16)
                sync.dma_start(wl_t[:], wld[:]).then_inc(s_in, 16)
                sync.wait_ge(s_z, 1)
                for rep in range(reps):
                    sync.wait_ge(s_g1c, (rep + 1) * NT)
                    sync.dma_start(
                        g1_shard[:].rearrange("(i p) f -> p i f", p=P),
                        gbuf[0][:]).then_inc(s_shard[0], 16)
                    if rep == 0:
                        sync.wait_ge(s_z, 2)
                    sync.wait_ge(s_epc[0], (rep + 1) * NT)
                    sync.dma_start(
                        g2_shard[:].rearrange("(i p) f -> p i f", p=P),
                        gbuf[1][:]).then_inc(s_shard[1], 16)
                    sync.wait_ge(s_epc[1], (rep + 1) * NT)
                    sync.dma_start(outd[:], out_buf[:]).then_inc(s_fin, 16)
                sync.wait_ge(s_fin, 16 * reps)

            @block.gpsimd
            def _(gpsimd):
                gpsimd.load_library(mlp)
                gpsimd.wait_ge(s_in, 112)
                GMAX = 8   # max chunks per dma_gather call (1024 idxs: HW limit)
                for rep in range(reps):
                  gpsimd.wait_ge(s_shard[0], 16 * (rep + 1))
                  if _abl == "nocoll":
                      gpsimd.sem_inc(s_cc, 1)
                  else:
                      gpsimd.collective_compute(
                          "AllGather", mybir.AluOpType.bypass,
                          ins=[g1_shard[:]], outs=[g1_full[:]],
                          replica_groups=[list(range(M))],
                      ).then_inc(s_cc, 1)
                  for l in (0, 1):
                    bmm = [rep * NCH, rep * NCH]
                    gpsimd.wait_ge(s_cc, 2 * rep + l + 1)
                    if l == 0 and rep > 0:
                        # msg ring handoff from previous rep's layer 1
                        gpsimd.wait_ge(s_mm[1], rep * NCH)
                    for w in range(NW):
                        if w >= 2:
                            gpsimd.wait_ge(
                                s_mm[l], rep * NCH + int(mm_win_end[w - 1]))
                        for b in range(NBUCK):
                            pc = int(piece_wb[w, b])
                            if pc == 0:
                                continue
                            for q0 in range(0, pc, GMAX):
                                qc = min(GMAX, pc - q0)
                                lo = int(win_start[w] + pboff[w, b]) + q0
                                n = qc * P
                                if _abl == "nogather":
                                    gpsimd.sem_inc(
                                        s_msgs[l][(w % 2) * NBUCK + b], 16)
                                    continue
                                gpsimd.dma_gather(
                                    msg_ring[:, w % 2,
                                             int(pboff[w, b]) + q0:
                                             int(pboff[w, b]) + q0 + qc, :],
                                    tables[l][b * BSZ:(b + 1) * BSZ, :],
                                    idx_t[:, lo * 8:lo * 8 + n // 16],
                                    n, n, FE,
                                ).then_inc(s_msgs[l][(w % 2) * NBUCK + b], 16)
                    if l == 0:
                        gpsimd.wait_ge(s_shard[1], 16 * (rep + 1))
                        if _abl == "nocoll":
                            gpsimd.sem_inc(s_cc, 1)
                        else:
                            gpsimd.collective_compute(
                                "AllGather", mybir.AluOpType.bypass,
                                ins=[g2_shard[:]], outs=[g2_full[:]],
                                replica_groups=[list(range(M))],
                            ).then_inc(s_cc, 1)

            @block.vector
            def _(vector):
                vector.memset(gbuf[0][:, :, HID:FE], 0).then_inc(s_z, 1)
                vector.wait_ge(s_in, 112)
                # g2_buf aliases xt: zero its pad columns only after phase A
                vector.wait_ge(s_mma, NT)
                vector.memset(gbuf[1][:, :, HID:FE], 0).then_inc(s_z, 1)
                for rep in range(reps):
                  for l in (0, 1):
                    if l == 1:
                        # oh ring handoff between layers
                        vector.wait_ge(s_mm[0], (rep + 1) * NCH)
                    elif rep > 0:
                        vector.wait_ge(s_mm[1], rep * NCH)
                    ndone = 0
                    for cc in range(NCH):
                        if cc % 16 == 0 and cc + 16 > RO:
                            vector.wait_ge(
                                s_mm[l], rep * NCH + min(cc + 16, NCH) - RO)
                        ts = vector.tensor_scalar(
                            out=oh_ring[:, cc % RO, :], in0=iota_t[:],
                            scalar1=cst[:, cc:cc + 1],
                            scalar2=cst[:, CT + cc:CT + cc + 1],
                            op0=mybir.AluOpType.is_equal,
                            op1=mybir.AluOpType.mult,
                        )
                        if cc % 8 == 7 or cc == NCH - 1:
                            ts.then_inc(s_oh[l], cc + 1 - ndone)
                            ndone = cc + 1

            @block.scalar
            def _(scalar):
                Relu = mybir.ActivationFunctionType.Relu
                Copy = mybir.ActivationFunctionType.Copy
                scalar.wait_ge(s_in, 112)
                for rep in range(reps):

                  for i in range(NT):
                    scalar.wait_ge(s_mma, rep * NT + i + 1)
                    scalar.activation(
                        out=gbuf[0][:, i, 0:HID], in_=ga_ps[i % 2], func=Copy,
                    ).then_inc(s_g1c, 1)

                  def relus(l, w, rep=rep):
                    for s in range(WSZ):
                        g = w * WSZ + s
                        scalar.wait_ge(
                            s_mm[l], rep * NCH + last_chunk[(w, s)] + 1)
                        if g >= ATR:
                            scalar.wait_ge(s_mm2[l], rep * NT + g - ATR + 1)
                        scalar.activation(
                            out=at_ring[:, g % ATR, :],
                            in_=seg_ps[:, g % RSEG, 0:P],
                            func=Relu,
                        ).then_inc(s_ep[l], 1)

                  def copies(l, w, rep=rep):
                    for s in range(WSZ):
                        g = w * WSZ + s
                        i = w + NW * s
                        scalar.wait_ge(s_mm2[l], rep * NT + g + 1)
                        if l == 0:
                            scalar.activation(
                                out=gbuf[1][:, i, 0:HID], in_=ga_ps[g % 2],
                                func=Copy,
                            ).then_inc(s_epc[0], 1)
                        else:
                            scalar.activation(
                                out=out_buf[:, i:i + 1],
                                in_=ga_ps[g % 2][:, 0:1], func=Copy,
                            ).then_inc(s_epc[1], 1)

                  for l in (0, 1):
                    if l == 0 and rep > 0:
                        scalar.wait_ge(s_mm2[1], rep * NT)
                    for w in range(NW):
                        relus(l, w)
                        if w >= 1:
                            copies(l, w - 1)
                    copies(l, NW - 1)

            @block.tensor
            def _(tensor):
                tensor.wait_ge(s_in, 112)
                GMAX = 8
                # per-layer total gather sub-calls per (parity, bucket) sem
                NCALLS = [0] * (2 * NBUCK)
                for w in range(NW):
                    for b in range(NBUCK):
                        pc = int(piece_wb[w, b])
                        NCALLS[(w % 2) * NBUCK + b] += (pc + GMAX - 1) // GMAX
                for rep in range(reps):
                  if rep > 0:
                    tensor.wait_ge(s_epc[1], rep * NT)
                    tensor.wait_ge(s_ep[1], rep * NT)
                  for i in range(NT):
                    gi = rep * NT + i
                    if gi >= 2:
                        tensor.wait_ge(s_g1c, gi - 1)
                    tensor.matmul(
                        out=ga_ps[i % 2],
                        lhsT=xt_t[:, i * P:(i + 1) * P],
                        rhs=w1_t[:, :],
                        start=True, stop=True,
                    ).then_inc(s_mma, 1)

                  def mm2s(l, w, rep=rep):
                    for s in range(WSZ):
                        g = w * WSZ + s
                        tensor.wait_ge(s_ep[l], rep * NT + g + 1)
                        if g >= 2:
                            tensor.wait_ge(s_epc[l], rep * NT + g - 1)
                        tensor.matmul(
                            out=ga_ps[g % 2] if l == 0 else ga_ps[g % 2][:, 0:1],
                            lhsT=at_ring[:, g % ATR, :],
                            rhs=w2_t[:, :] if l == 0 else wl_t[:, :],
                            start=True, stop=True,
                        ).then_inc(s_mm2[l], 1)

                  for l in (0, 1):
                    if l == 0:
                        tensor.wait_ge(s_g1c, (rep + 1) * NT)
                    else:
                        tensor.wait_ge(s_epc[0], (rep + 1) * NT)
                    ncall = [0] * (2 * NBUCK)
                    waited = set()
                    for cc, (w, s, b, c) in enumerate(sched):
                        g = w * WSZ + s
                        if b >= 0 and (w, b) not in waited:
                            waited.add((w, b))
                            pc = int(piece_wb[w, b])
                            if pc > 0:
                                sl = (w % 2) * NBUCK + b
                                ncall[sl] += (pc + GMAX - 1) // GMAX
                                tensor.wait_ge(
                                    s_msgs[l][sl],
                                    (rep * NCALLS[sl] + ncall[sl]) * 16)
                        if cc % 8 == 0:
                            tensor.wait_ge(
                                s_oh[l], rep * NCH + min(cc + 8, NCH))
                        st = first_chunk[(w, s)] == cc
                        if st and g >= RSEG:
                            tensor.wait_ge(s_ep[l], rep * NT + g - RSEG + 1)
                        if b < 0:
                            lhs = gbuf[l][:, w + NW * s, 0:HID]
                        else:
                            moff = int(gwoff[w, s, b]) + c
                            lhs = msg_ring[:, w % 2, moff, 0:HID]
                        tensor.matmul(
                            out=seg_ps[:, g % RSEG, 0:P],
                            lhsT=lhs,
                            rhs=oh_ring[:, cc % RO, :],
                            start=st, stop=last_chunk[(w, s)] == cc,
                        ).then_inc(s_mm[l], 1)
                        if cc == int(mm_win_end[w + 1]) - 1 and w >= 1:
                            mm2s(l, w - 1)
                    mm2s(l, NW - 1)

    nc.compile()
    return nc


def _run(nc, in_maps):
    from concourse.bass_utils import run_bass_kernel_spmd
    return run_bass_kernel_spmd(nc, in_maps, core_ids=list(range(M)))


class _Runner:
    """Persistent compiled executable + device-resident inputs.

    Mimics bass2jax.run_bass_via_pjrt but keeps the jitted shard_map and the
    device arrays alive so repeat executions only pay dispatch + exec.
    """

    def __init__(self, nc, in_maps):
        import jax
        import numpy as _np
        import concourse.mybir as mybir
        from jax.sharding import Mesh, PartitionSpec
        from jax.experimental.shard_map import shard_map
        from concourse import bass2jax

        bass2jax.install_neuronx_cc_hook()
        self.jax = jax
        pname = nc.partition_id_tensor.name if nc.partition_id_tensor else None
        in_names, out_names, out_avals, zero_outs = [], [], [], []
        for alloc in nc.m.functions[0].allocations:
            if not isinstance(alloc, mybir.MemoryLocationSet):
                continue
            name = alloc.memorylocations[0].name
            if alloc.kind == "ExternalInput":
                if name != pname:
                    in_names.append(name)
            elif alloc.kind == "ExternalOutput":
                shape = tuple(alloc.tensor_shape)
                dtype = mybir.dt.np(alloc.dtype)
                out_names.append(name)
                out_avals.append(jax.core.ShapedArray(shape, dtype))
                zero_outs.append(_np.zeros(shape, dtype))
        n_params = len(in_names)
        all_in = in_names + out_names
        if pname is not None:
            all_in.append(pname)
        self.out_names = out_names
        self.out_avals = out_avals

        def _body(*args):
            operands = list(args)
            if pname is not None:
                operands.append(bass2jax.partition_id_tensor())
            return tuple(bass2jax._bass_exec_p.bind(
                *operands, out_avals=tuple(out_avals), in_names=tuple(all_in),
                out_names=tuple(out_names), lowering_input_output_aliases=(),
                sim_require_finite=True, sim_require_nnan=True, nc=nc))

        devices = jax.devices()[:M]
        mesh = Mesh(_np.asarray(devices), ("core",))
        nio = n_params + len(out_names)
        self.fn = jax.jit(shard_map(
            _body, mesh=mesh, in_specs=(PartitionSpec("core"),) * nio,
            out_specs=(PartitionSpec("core"),) * len(out_names),
            check_rep=False),
            donate_argnums=tuple(range(n_params, nio)), keep_unused=True)
        self.sharding = jax.sharding.NamedSharding(mesh, PartitionSpec("core"))
        concat_in = [
            _np.concatenate([_np.asarray(in_maps[c][nm]) for c in range(M)],
                            axis=0)
            for nm in in_names]
        self.dev_in = [jax.device_put(a, self.sharding) for a in concat_in]
        self.zero_outs = zero_outs

    def __call__(self):
        zs = [self.jax.device_put(
            self.jax.numpy.zeros((M * z.shape[0], *z.shape[1:]), z.dtype),
            self.sharding) for z in self.zero_outs]
        outs = self.fn(*self.dev_in, *zs)
        self.jax.block_until_ready(outs)
        return outs

    def results(self):
        import numpy as _np
        outs = self()
        return [
            {nm: _np.asarray(outs[i]).reshape(
                M, *self.out_avals[i].shape)[c]
             for i, nm in enumerate(self.out_names)}
            for c in range(M)]


def kernel(x, edge_index, edge_weight, W1, b1, W2, b2, Wl, bl):
    W1 = np.asarray(W1, np.float32)
    b1 = np.asarray(b1, np.float32)
    W2 = np.asarray(W2, np.float32)
    b2 = np.asarray(b2, np.float32)
    Wl = np.asarray(Wl, np.float32)
    bl = np.asarray(bl, np.float32)
    if np.any(b1 != 0) or np.any(b2 != 0):
        return _kernel_numpy(x, edge_index, edge_weight, W1, b1, W2, b2, Wl, bl)

    prep = _host_prep(x, edge_index, edge_weight)
    in_maps = _in_maps(prep, W1, W2, Wl)
    nc = _get_nc(prep)
    res = _run(nc, in_maps)
    return _assemble(res.results, prep, bl)


def _get_nc(prep, reps=1):
    key = (prep["CT"], reps, prep["profile"].tobytes())
    if key not in _cache:
        _cache[key] = _build_nc(prep["profile"], prep["CT"], reps=reps)
    return _cache[key]


def _in_maps(prep, W1, W2, Wl):
    w1b = np.ascontiguousarray(W1).astype(ml_dtypes.bfloat16)
    w2b = np.ascontiguousarray(W2).astype(ml_dtypes.bfloat16)
    wlb = np.ascontiguousarray(Wl).astype(ml_dtypes.bfloat16)
    iota_b = np.tile(np.arange(P, dtype=np.float32), (P, 1)).astype(
        ml_dtypes.bfloat16)
    return [{
        "xt": prep["xt"][c], "idx_w": prep["idx_w"][c], "cst": prep["cst"][c],
        "iotab": iota_b, "w1": w1b, "w2": w2b, "wl": wlb,
    } for c in range(M)]


def _assemble(results, prep, bl):
    out_full = np.zeros(NTILE * P + P, np.float32)
    assign = prep["assign"]
    for c in range(M):
        o = np.asarray(results[c]["outd"])
        for i in range(NT):
            t = assign[c, i]
            if t >= 0:
                out_full[t * P:(t + 1) * P] = o[:, i]
    return (out_full[:N] + np.float32(np.asarray(bl).reshape(-1)[0])).astype(
        np.float32)


def _kernel_numpy(x, edge_index, edge_weight, W1, b1, W2, b2, Wl, bl):
    x = np.asarray(x, dtype=np.float32)
    ei = np.asarray(edge_index).astype(np.int64)
    ew_in = np.asarray(edge_weight, dtype=np.float32)
    loop = np.arange(N, dtype=np.int64)
    src = np.concatenate([ei[0], loop])
    dst = np.concatenate([ei[1], loop])
    ew = np.concatenate([ew_in, np.ones(N, dtype=np.float32)])
    deg = np.bincount(dst, weights=ew, minlength=N)
    dinv = np.where(deg > 0, 1.0 / np.sqrt(deg), 0.0).astype(np.float32)
    norm = (dinv[src] * ew * dinv[dst]).astype(np.float32)

    def prop(h):
        msg = h[src] * norm[:, None]
        out = np.zeros((N, h.shape[1]), np.float32)
        np.add.at(out, dst, msg)
        return out

    h = np.maximum(prop(x @ W1) + b1, 0.0)
    h = np.maximum(prop(h @ W2) + b2, 0.0)
    return (h @ Wl + bl).squeeze(-1).astype(np.float32)

